# revision 2
# baseline (speedup 1.0000x reference)
"""Causal self-attention (GQA + RoPE) for TRN2, sharded over 8 NeuronCores.

Sharding: tensor-parallel over heads. Each core owns 4 query heads and 1 KV
head (H=32, HKV=8 -> group size 4). Column-parallel q/k/v projections,
row-parallel o_proj; the final all-reduce over the 8 partial [T, D] outputs
happens on the host after the gather.

Performance design (377.8us f32r baseline -> 214.4us bf16 -> this version):
  - Hybrid fp8(e4m3)+bf16 numerics: strip 0 (queries/keys 0-511) runs
    entirely in bf16; strips 1-3 run their projections, attn@v and o_proj
    as fp8 DoubleRow matmuls (2 contraction chunks per instruction, ~2x PE
    throughput). The max error is concentrated in the first ~32 rows
    (few attention keys -> no averaging), which stay on the bf16 path;
    rows >= 512 average over >= 512 keys so fp8 noise washes out
    (measured 5.9e-3 scaled max err vs the 2e-2 gate).
  - Scores matmuls stay bf16 (no DoubleRow win at contraction 64); exp
    output is written fp8 directly into paired ex tiles [128, 2, S] that
    feed the DoubleRow attn@v.
  - Software-pipelined emission with lookahead 3, projection/o_proj PE
    gap fillers, split-lifetime PSUM rings, DVE softmax reciprocal and
    PE broadcast -- as in the bf16 version (see git history for the
    full rationale of each).
Layout (all on-chip tensors keep the contraction dim on partitions):
  - x is uploaded pre-transposed as xT [D, T] (bf16, strip 0) and xT8
    (fp8, strips 1-3); projections produce qT/kT/vT [d, t] in PSUM.
  - RoPE uses a host-side permutation of the head dim into [even-pairs |
    odd-pairs] halves so the pair rotation becomes: out = q*cos +
    swap64(q*sin'), where swap64 swaps 32-row halves within each 64-row
    head block (SBUF->SBUF DMAs); sin' carries the sign pattern.
  - Scores are computed transposed (scoresT [s, t]) so the softmax
    denominator comes out of the attn@v matmul for free via the ones
    column in the v stationary operand; causal masking of diagonal blocks
    is a post-exp gpsimd affine_select; for fp8 pairs the ktile-1 gap
    columns of diagonal pairs are memset to zero.
"""

import math

import numpy as np

import concourse.bass as bass
import concourse.mybir as mybir
import concourse.tile as tile
from concourse import bacc
from concourse.masks import make_identity

D = 2048
H = 32
HKV = 8
HD = 64
T = 2048
NCORES = 8
HPC = H // NCORES        # 4 query heads per core
QC = HPC * HD            # 256 q dims per core
ROPE_BASE = 10000.0
S = 512                  # t-strip / moving-operand width
NSTRIP = T // S          # 4
KC = D // 128            # 16 contraction chunks

F32 = mybir.dt.float32
BF16 = mybir.dt.bfloat16
FP8 = mybir.dt.float8e4
DR = mybir.MatmulPerfMode.DoubleRow


def _build_kernel(debug=False):
    nc = bacc.Bacc("TRN2", target_bir_lowering=False, debug=False,
                   num_devices=NCORES)

    xT = nc.dram_tensor("xT", [D, S], BF16, kind="ExternalInput").ap()
    xT8 = nc.dram_tensor("xT8", [D, T], FP8, kind="ExternalInput").ap()
    wqT = nc.dram_tensor("wqT", [D, QC], BF16, kind="ExternalInput").ap()
    wq8T = nc.dram_tensor("wq8T", [D, QC], FP8, kind="ExternalInput").ap()
    wkvT = nc.dram_tensor("wkvT", [D, 128], BF16, kind="ExternalInput").ap()
    wkv8T = nc.dram_tensor("wkv8T", [D, 128], FP8,
                           kind="ExternalInput").ap()
    woT = nc.dram_tensor("woT", [QC, D], BF16, kind="ExternalInput").ap()
    wo8T = nc.dram_tensor("wo8T", [QC, D], FP8, kind="ExternalInput").ap()
    cosT = nc.dram_tensor("cosT", [128, T], BF16,
                          kind="ExternalInput").ap()
    sinT = nc.dram_tensor("sinT", [128, T], BF16,
                          kind="ExternalInput").ap()
    out = nc.dram_tensor("out", [T, D], BF16, kind="ExternalOutput").ap()

    with tile.TileContext(nc) as tc:
        with (
            tc.tile_pool(name="consts", bufs=1) as consts,
            tc.tile_pool(name="persist", bufs=1) as persist,
            tc.tile_pool(name="xa", bufs=1) as xap,
            tc.tile_pool(name="xa8", bufs=3) as xap8,
            tc.tile_pool(name="rtmp", bufs=6) as rtmp,
            tc.tile_pool(name="swp", bufs=4) as swp,
            tc.tile_pool(name="vtmp", bufs=2) as vtmp,
            tc.tile_pool(name="expp", bufs=6) as expp,
            tc.tile_pool(name="ytn", bufs=6) as ytnp,
            tc.tile_pool(name="outst", bufs=3) as outst,
            tc.tile_pool(name="dn", bufs=4) as dnp,
            tc.tile_pool(name="mmS", bufs=2, space="PSUM") as mmS,
            tc.tile_pool(name="mmP", bufs=2, space="PSUM") as mmP,
            tc.tile_pool(name="mmO", bufs=2, space="PSUM") as mmO,
            tc.tile_pool(name="ytps", bufs=2, space="PSUM") as ytps,
        ):
            # ---- constants; DMA issue order interleaved per k-chunk so the
            # first projection matmul only waits on chunk 0 of wq/x ----
            wq_sb = consts.tile([128, KC, QC], BF16)
            wq8_sb = consts.tile([128, KC, QC], FP8)
            wkv_sb = consts.tile([128, KC, 128], BF16)
            wkv8_sb = consts.tile([128, KC, 128], FP8)
            xa_strips = {}
            xa_tiles = {}

            def load_xa(strip):
                # strips >= 1 load the fp8 copy of x
                t0 = strip * S
                xt = xap8.tile([128, KC, S], FP8, tag="xa8",
                               name=f"xa{strip}")
                nc.sync.dma_start(
                    out=xt,
                    in_=xT8.rearrange("(c p) t -> p c t", p=128)[:, :,
                                                                t0:t0 + S])
                xa_strips[strip] = [xt[:, kc, :] for kc in range(KC)]
                xa_tiles[strip] = xt

            wqT_r = wqT.rearrange("(c p) q -> p c q", p=128)
            xT_r = xT.rearrange("(c p) t -> p c t", p=128)
            # startup loads spread across 4 queues so SWDGE descriptor
            # generation (3-7us per big rearranged DMA) runs in parallel
            # staircase the entry loads in consumption order across three
            # queues so projection chunks land incrementally instead of in
            # two big all-at-once waves
            xt0 = xap.tile([128, KC, S], BF16, tag="xa", name="xa0")
            nc.sync.dma_start(out=xt0[:, 0:4, :], in_=xT_r[:, 0:4, 0:S])
            nc.scalar.dma_start(out=wq_sb[:, 0:4, :], in_=wqT_r[:, 0:4, :])
            nc.gpsimd.dma_start(
                out=wkv_sb, in_=wkvT.rearrange("(c p) q -> p c q", p=128))
            nc.sync.dma_start(out=xt0[:, 4:8, :], in_=xT_r[:, 4:8, 0:S])
            nc.scalar.dma_start(out=wq_sb[:, 4:10, :], in_=wqT_r[:, 4:10, :])
            nc.sync.dma_start(out=xt0[:, 8:12, :], in_=xT_r[:, 8:12, 0:S])
            nc.gpsimd.dma_start(out=xt0[:, 12:KC, :],
                                in_=xT_r[:, 12:KC, 0:S])
            nc.scalar.dma_start(out=wq_sb[:, 10:KC, :],
                                in_=wqT_r[:, 10:KC, :])
            xa_strips[0] = [xt0[:, kc, :] for kc in range(KC)]
            xa_tiles[0] = xt0
            cs_c = consts.tile([128, T], BF16)
            cs_s = consts.tile([128, T], BF16)
            nc.scalar.dma_start(out=cs_c, in_=cosT)
            nc.sync.dma_start(out=cs_s, in_=sinT)
            load_xa(1)
            # fp8 weights (needed from strip 1's projection filler on)
            nc.gpsimd.dma_start(
                out=wq8_sb, in_=wq8T.rearrange("(c p) q -> p c q", p=128))
            nc.gpsimd.dma_start(
                out=wkv8_sb, in_=wkv8T.rearrange("(c p) q -> p c q", p=128))
            wo_sb = consts.tile([128, 2, D], BF16)
            wo8_sb = consts.tile([128, 2, D], FP8)
            ident_f = consts.tile([128, 128], F32)
            make_identity(nc, ident_f)
            ident = consts.tile([128, 128], BF16)
            nc.vector.tensor_copy(ident, ident_f)
            # PE warmup: keep the array continuously busy while the first
            # input DMAs land so the p-state ramp is done by the first
            # projection matmul (fp32 on purpose: 4 cyc/row keeps the array
            # busy longer per instruction)
            warm_ps = mmO.tile([128, 512], F32, tag="mmO", name="warm")
            junk = consts.tile([128, 512], F32)
            nc.vector.memset(junk, 1.0)
            for w in range(2):
                nc.tensor.matmul(
                    warm_ps, ident_f, junk,
                    start=True, stop=True, skip_group_check=True)
            ones_b = consts.tile([128, 64], BF16)
            nc.vector.memset(ones_b, 1.0)

            # persistent activations
            qT = [persist.tile([128, T], BF16, tag=f"qT{i}", name=f"qT{i}")
                  for i in range(2)]
            # k duplicated on both partition halves so each q head can use
            # a stationary slice whose base partition matches its rhs base
            kT = persist.tile([128, T], BF16)
            # vaug columns: [v(64) | ones]; the ones column makes the
            # softmax denominator fall out of the attn@v matmul. bf16 copy
            # only for strip 0's chunks; fp8 copy (stride 80 to keep the
            # DoubleRow ktile step 16B-aligned) for all chunks.
            vaug = persist.tile([128, 4, 65], BF16)
            vaug8 = persist.tile([128, 4 * NSTRIP, 80], FP8)
            ones_col = consts.tile([128, 4 * NSTRIP, 1], F32)
            nc.vector.memset(ones_col, 1.0)
            nc.vector.tensor_copy(vaug[:, :, 64:65], ones_col[:, 0:4, :])
            nc.vector.tensor_copy(vaug8[:, :, 64:65], ones_col)

            def proj_filler(strip, dense=False, part=None):
                """Yield closures, each emitting one PE op of this strip's
                q/kv projection; rope/evict DVE work rides along after the
                last matmul of each accumulation group. The dense (pre-loop)
                call borrows the then-idle mmS ring for the q tiles so the
                three groups are not serialized by the 1-slot mmP ring.
                Strips >= 1 use fp8 DoubleRow (2 k-chunks per matmul)."""
                use8 = strip >= 1
                qpool, qtag = (mmP, "mmP")
                kvpool, kvtag = (mmS, "mmS") if dense else (mmP, "mmP")
                t0 = strip * S
                tsl = slice(t0, t0 + S)
                xa = xa_strips[strip]
                xat = xa_tiles[strip]
                wq_s = wq8_sb if use8 else wq_sb
                wkv_s = wkv8_sb if use8 else wkv_sb
                NCH = KC // 2 if use8 else KC

                def q_matmul(ps, c, lo, hi):
                    if use8:
                        nc.tensor.matmul(
                            ps, wq_s[:, 2 * c:2 * c + 2, lo:hi],
                            xat[:, 2 * c:2 * c + 2, :], start=(c == 0),
                            stop=(c == NCH - 1), perf_mode=DR)
                    else:
                        nc.tensor.matmul(
                            ps, wq_s[:, c, lo:hi], xa[c], start=(c == 0),
                            stop=(c == NCH - 1))

                def kv_matmul(ps, c):
                    if use8:
                        nc.tensor.matmul(
                            ps, wkv_s[:, 2 * c:2 * c + 2, :],
                            xat[:, 2 * c:2 * c + 2, :], start=(c == 0),
                            stop=(c == NCH - 1), perf_mode=DR)
                    else:
                        nc.tensor.matmul(
                            ps, wkv_s[:, c, :], xa[c], start=(c == 0),
                            stop=(c == NCH - 1))

                def rope_q(hp, pq):
                    qc = rtmp.tile([128, S], F32, tag="rtmp",
                                   name=f"qc{strip}{hp}")
                    qs = rtmp.tile([128, S], F32, tag="rtmp",
                                   name=f"qs{strip}{hp}")
                    nc.vector.tensor_mul(qs, pq, cs_s[:, tsl])
                    sw = swp.tile([128, S], F32, tag="swp",
                                  name=f"sw{strip}{hp}")
                    for b in range(2):
                        nc.sync.dma_start(
                            out=sw[b * 64:b * 64 + 32, :],
                            in_=qs[b * 64 + 32:b * 64 + 64, :])
                        nc.sync.dma_start(
                            out=sw[b * 64 + 32:b * 64 + 64, :],
                            in_=qs[b * 64:b * 64 + 32, :])
                    nc.vector.tensor_mul(qc, pq, cs_c[:, tsl])
                    nc.vector.tensor_add(qT[hp][:, tsl], qc, sw)

                if part != "b":
                    pq0 = qpool.tile([128, S], F32, tag=qtag,
                                     name=f"pq{strip}_0")
                    for kc in range(NCH):
                        def mk(kc=kc):
                            q_matmul(pq0, kc, 0, 128)
                            if kc == NCH - 1:
                                rope_q(0, pq0)
                        yield mk

                def rope_kv():
                    kc_t = rtmp.tile([128, S], F32, tag="rtmp",
                                     name=f"kc{strip}")
                    ks_t = rtmp.tile([128, S], F32, tag="rtmp",
                                     name=f"ks{strip}")
                    nc.vector.tensor_mul(
                        ks_t[0:64, :], pkv[0:64, :], cs_s[0:64, tsl])
                    swk = swp.tile([128, S], F32, tag="swp",
                                   name=f"swk{strip}")
                    nc.gpsimd.dma_start(out=swk[0:32, :], in_=ks_t[32:64, :])
                    nc.gpsimd.dma_start(out=swk[32:64, :], in_=ks_t[0:32, :])
                    nc.vector.tensor_mul(
                        kc_t[0:64, :], pkv[0:64, :], cs_c[0:64, tsl])
                    nc.vector.tensor_add(
                        kT[0:64, tsl], kc_t[0:64, :], swk[0:64, :])
                    nc.gpsimd.dma_start(out=kT[64:128, tsl], in_=kT[0:64, tsl])
                    vt_s = vtmp.tile([128, S], BF16, tag="vtmp",
                                     name=f"vt{strip}")
                    nc.vector.tensor_copy(vt_s[64:128, :], pkv[64:128, :])
                    return vt_s

                state = {}
                if part != "b":
                    pkv = kvpool.tile([128, S], F32, tag=kvtag,
                                      name=f"pkv{strip}")
                    for kc in range(NCH):
                        def mk(kc=kc, pkv=pkv):
                            kv_matmul(pkv, kc)
                            if kc == NCH - 1:
                                state["vt_s"] = rope_kv()
                        yield mk

                if part != "a":
                    pq1 = qpool.tile([128, S], F32, tag=qtag,
                                     name=f"pq{strip}_1")
                    for kc in range(NCH):
                        def mk(kc=kc):
                            q_matmul(pq1, kc, 128, 256)
                            if kc == NCH - 1:
                                rope_q(1, pq1)
                        yield mk
                if part == "b":
                    return
                for n in range(4):
                    def mk(n=n):
                        pt = mmO.tile([128, 64], BF16, tag="mmO",
                                      name=f"pt{strip}{n}")
                        nc.tensor.transpose(
                            pt, state["vt_s"][64:128, n * 128:(n + 1) * 128],
                            ident[64:128, 64:128])
                        if strip == 0:
                            nc.vector.tensor_copy(vaug[:, n, 0:64], pt)
                        nc.vector.tensor_copy(
                            vaug8[:, strip * 4 + n, 0:64], pt)
                    yield mk

            def oproj_filler(strip, ytn, evict_alt=False):
                """Yield closures, each emitting one o_proj matmul; DVE
                eviction into the packed row buffer rides after each group's
                stop, one store DMA per tsub. Strip 0: bf16, 2 matmuls per
                (tsub, n). Strips >= 1: one fp8 DoubleRow matmul (ytn is
                then the packed [128, 2, S] fp8 tile)."""
                use8 = strip >= 1
                t0 = strip * S
                for tsub in range(4):
                    trow = t0 + tsub * 128
                    ot = outst.tile([128, D], BF16, tag="out",
                                    name=f"ot{strip}{tsub}")
                    for n in range(4):
                        po = mmO.tile([128, S], F32, tag="mmO",
                                      name=f"po{strip}{tsub}{n}")

                        def evict(po, n, tsub, trow, ot):
                            if evict_alt and n % 2 == 1:
                                nc.scalar.copy(
                                    ot[:, n * S:(n + 1) * S], po)
                            else:
                                nc.vector.tensor_copy(
                                    ot[:, n * S:(n + 1) * S], po)
                            if evict_alt:
                                eng = (nc.gpsimd if n % 2 == 0
                                       else nc.sync)
                                eng.dma_start(
                                    out=out[trow:trow + 128,
                                            n * S:(n + 1) * S],
                                    in_=ot[:, n * S:(n + 1) * S])
                            elif n == 3:
                                nc.gpsimd.dma_start(
                                    out=out[trow:trow + 128, :],
                                    in_=ot)

                        if use8:
                            def mk(po=po, tsub=tsub, n=n, trow=trow, ot=ot):
                                nc.tensor.matmul(
                                    po,
                                    ytn[:, :, tsub * 128:(tsub + 1) * 128],
                                    wo8_sb[:, :, n * S:(n + 1) * S],
                                    start=True, stop=True, perf_mode=DR,
                                    skip_group_check=True)
                                evict(po, n, tsub, trow, ot)
                            yield mk
                        else:
                            for c in range(2):
                                def mk(po=po, c=c, tsub=tsub, n=n,
                                       trow=trow, ot=ot):
                                    nc.tensor.matmul(
                                        po,
                                        ytn[c][:,
                                               tsub * 128:(tsub + 1) * 128],
                                        wo_sb[:, c, n * S:(n + 1) * S],
                                        start=(c == 0), stop=(c == 1),
                                        skip_group_check=True)
                                    if c == 1:
                                        evict(po, n, tsub, trow, ot)
                                yield mk

            def run_filler(filler, frac):
                """Emit pending filler ops; frac is how many to emit now."""
                import itertools
                for fn in itertools.islice(filler, frac):
                    fn()

            # strip 0 projection runs dense (nothing to overlap with)
            for fn in proj_filler(0, dense=True):
                fn()

            ytn_strips = {}
            fillers = []  # queue of generators feeding PE gap-filler ops

            PROJ_OPS8 = 3 * (KC // 2) + 4      # fp8 proj filler op count
            OPROJ_OPS = {0: 32, 1: 16, 2: 16, 3: 16}

            for strip in range(NSTRIP):
                use8 = strip >= 1
                t0 = strip * S
                n_sc = (strip + 1) * 4
                if use8:
                    # packed fp8 [128, 2, S]: dim1 = o_proj contraction chunk
                    ytn = ytnp.tile([128, 2, S], FP8, tag="ytn",
                                    name=f"ytn{strip}")
                else:
                    ytn = [ytnp.tile([128, S], BF16, tag="ytn",
                                     name=f"ytn{strip}{i}") for i in range(2)]
                ytn_strips[strip] = ytn

                if strip == 0:
                    nc.gpsimd.dma_start(
                        out=wo_sb,
                        in_=woT.rearrange("(c p) n -> p c n", p=128))
                    nc.scalar.dma_start(
                        out=wo8_sb,
                        in_=wo8T.rearrange("(c p) n -> p c n", p=128))
                if strip + 1 < NSTRIP:
                    if strip + 1 not in xa_strips:
                        load_xa(strip + 1)
                    if strip + 2 < NSTRIP and strip + 2 not in xa_strips:
                        load_xa(strip + 2)
                    fillers.append(proj_filler(strip + 1))
                if strip - 1 >= 0:
                    og = oproj_filler(strip - 1, ytn_strips[strip - 1])
                    if strip == NSTRIP - 2:
                        # give half of this o_proj to the ACT-bound final
                        # strip, where PE slots are free
                        import itertools
                        fillers.append(
                            itertools.islice(og, OPROJ_OPS[strip - 1] // 2))
                        deferred_oproj = og
                    else:
                        fillers.append(og)
                if strip == NSTRIP - 1:
                    fillers.insert(0, deferred_oproj)

                n_chunks = HPC * n_sc
                pending = PROJ_OPS8 if strip + 1 < NSTRIP else 0
                if strip - 1 >= 0:
                    pending += (OPROJ_OPS[strip - 1] // 2
                                if strip >= NSTRIP - 2
                                else OPROJ_OPS[strip - 1])
                if strip == 0:
                    gate = 5
                elif strip + 1 < NSTRIP:
                    gate = n_chunks // 3
                else:
                    gate = 0
                per_chunk = (-(-pending // max(n_chunks - gate, 1))
                             if pending else 0)

                import itertools
                filler_iter = itertools.chain(*fillers)
                fillers = [filler_iter]

                # chunk sequence across heads; even heads (lo=0) first: odd
                # heads need the kT half-dup DMA which lands a bit later
                horder = (1, 3, 0, 2) if strip == NSTRIP - 1 else (0, 2, 1, 3)
                seq = [(h, j) for h in horder for j in range(n_sc)]
                pq1_iter = None

                ex_pairs = {}  # h -> current fp8 ex pair tile

                def emit_scores(h, j):
                    """Scores matmul + diag mask + exp; returns state the
                    deferred attn@v needs."""
                    hp, lo = h // 2, (h % 2) * 64
                    o = max(j * 128 - t0, 0)
                    if strip == NSTRIP - 1 and (h + j) % 2 == 1:
                        # final strip has no projection fillers, so the mmP
                        # ring is idle -- alternating pools doubles the
                        # effective scores-ring depth and unchains the
                        # chunk period from the exp latency
                        ps_sc = mmP.tile([128, S], F32, tag="mmP",
                                         name=f"s{strip}{h}{j}")
                    else:
                        ps_sc = mmS.tile([128, S], F32, tag="mmS",
                                         name=f"s{strip}{h}{j}")
                    diag = j * 128 - t0 >= 0
                    nc.tensor.matmul(
                        ps_sc[:, o:S],
                        kT[lo:lo + 64, j * 128:(j + 1) * 128],
                        qT[hp][lo:lo + 64, t0 + o:t0 + S],
                        start=True, stop=True, skip_group_check=True)
                    if use8:
                        kt = j % 2
                        if kt == 0:
                            ex = expp.tile([128, 2, S], FP8, tag="exp",
                                           name=f"e{strip}{h}{j}")
                            ex_pairs[h] = (ex, o)
                        ex, o0 = ex_pairs[h]
                        nc.scalar.activation(
                            ex[:, kt, o:S], ps_sc[:, o:S],
                            mybir.ActivationFunctionType.Exp,
                            scale=1.0 / math.sqrt(HD))
                        if kt == 1 and o > o0:
                            # ktile-1 gap columns of a diagonal pair: the
                            # DoubleRow matmul reads them; exp never wrote
                            nc.vector.memset(ex[:, 1, o0:o], 0.0)
                        if diag:
                            nc.gpsimd.affine_select(
                                out=ex[:, kt, o:o + 128],
                                in_=ex[:, kt, o:o + 128],
                                pattern=[[1, 128]], base=0,
                                channel_multiplier=-1,
                                compare_op=mybir.AluOpType.is_ge, fill=0.0)
                        return (h, j, o0, ex)
                    else:
                        ex = expp.tile([128, S], BF16, tag="exp",
                                       name=f"e{strip}{h}{j}")
                        nc.scalar.activation(
                            ex[:, o:S], ps_sc[:, o:S],
                            mybir.ActivationFunctionType.Exp,
                            scale=1.0 / math.sqrt(HD))
                        if diag:
                            nc.gpsimd.affine_select(
                                out=ex[:, o:o + 128], in_=ex[:, o:o + 128],
                                pattern=[[1, 128]], base=0,
                                channel_multiplier=-1,
                                compare_op=mybir.AluOpType.is_ge, fill=0.0)
                        return (h, j, o, ex)

                def emit_attnv(st, yt_ps):
                    h, j, o, ex = st
                    if use8:
                        if j % 2 == 0:
                            return  # pair issues on the odd chunk
                        p = j // 2
                        nc.tensor.matmul(
                            yt_ps[0:65, o:S],
                            vaug8[:, j - 1:j + 1, 0:65],
                            ex[:, :, o:S],
                            start=(p == 0), stop=(p == n_sc // 2 - 1),
                            perf_mode=DR, skip_group_check=True)
                    else:
                        nc.tensor.matmul(
                            yt_ps[0:65, o:S], vaug[:, j, :], ex[:, o:S],
                            start=(j == 0), stop=(j == n_sc - 1),
                            skip_group_check=True)

                def emit_normalize_a(h, yt_ps):
                    """Stage A (right after the closing attn@v): reciprocal
                    of the denom row on the DVE, cast to bf16 for the PE
                    broadcast matmul."""
                    dn = dnp.tile([128, S], F32, tag="dnr",
                                  bufs=2, name=f"dnr{strip}{h}")
                    # the custom op mis-executes on 1-row slices; run it on
                    # the full tile (rows != 64 are unused garbage)
                    nc.vector.reciprocal_approx_fast(out=dn, in_=yt_ps)
                    dnb = dnp.tile([128, S], BF16, tag="dnb",
                                   bufs=2, name=f"dnb{strip}{h}")
                    nc.vector.tensor_copy(dnb[64:65, :], dn[64:65, :])
                    return dnb

                def emit_normalize_b(h, yt_ps, dn):
                    """Stage B (two chunks later): PE outer-product
                    broadcast of the 1/denom row across partitions, then the
                    DVE normalize mul; odd heads land on partitions 64-127
                    of ytn via a gpsimd copy."""
                    hp, odd = h // 2, h % 2
                    bc_ps = mmO.tile([64, S], F32, tag="mmO",
                                     name=f"bp{strip}{h}")
                    nc.tensor.matmul(
                        bc_ps, ones_b[64:65, :], dn[64:65, :],
                        start=True, stop=True, skip_group_check=True)
                    bc_t = dnp.tile([128, S], F32, tag="dn",
                                    name=f"bc{strip}{h}")
                    nc.vector.tensor_copy(bc_t[0:64, :], bc_ps)
                    if use8:
                        ydst = ytn[0:64, hp, :]
                    else:
                        ydst = ytn[hp][0:64, :]
                    if not odd:
                        nc.vector.tensor_mul(
                            ydst, yt_ps[0:64, :], bc_t[0:64, :])
                    else:
                        ntmp = dnp.tile([128, S], FP8 if use8 else BF16,
                                        tag="ntmp", bufs=2,
                                        name=f"nt{strip}{h}")
                        nc.vector.tensor_mul(
                            ntmp[0:64, :], yt_ps[0:64, :], bc_t[0:64, :])
                        if use8:
                            nc.gpsimd.dma_start(
                                out=ytn[64:128, hp, :], in_=ntmp[0:64, :])
                        else:
                            nc.gpsimd.dma_start(
                                out=ytn[hp][64:128, :], in_=ntmp[0:64, :])

                LA = 3  # scores run LA chunks ahead of attn@v
                pending_ops = []  # (due_idx, closure)
                inflight = []     # [(st, yt)] scores awaiting attn@v
                yt_cur = None

                def retire(idx):
                    st, cyt = inflight.pop(0)
                    emit_attnv(st, cyt)
                    if st[1] == n_sc - 1:  # closing chunk of a head
                        ch = st[0]
                        dn = emit_normalize_a(ch, cyt)
                        pending_ops.append(
                            (idx + 3,
                             lambda ch=ch, cyt=cyt, dn=dn:
                             emit_normalize_b(ch, cyt, dn)))

                for idx, (h, j) in enumerate(seq):
                    for due, fn in [p for p in pending_ops if p[0] <= idx]:
                        fn()
                    pending_ops = [p for p in pending_ops if p[0] > idx]
                    if j == 0:
                        yt_cur = ytps.tile([128, S], F32, tag="yt",
                                           name=f"yt{strip}{h}")
                    st = emit_scores(h, j)
                    inflight.append((st, yt_cur))
                    if len(inflight) > LA:
                        retire(idx)
                        if pq1_iter is not None and idx >= 1:
                            run_filler(pq1_iter, 8)
                        if idx >= gate:
                            run_filler(filler_iter, per_chunk)

                # close out the strip
                idx = len(seq)
                while inflight:
                    retire(idx)
                    idx += 1
                for due, fn in sorted(pending_ops):
                    fn()

                # drain any leftover filler before the next strip
                for fn in filler_iter:
                    fn()
                fillers = []

            # last strip's o_proj runs dense at the tail
            for fn in oproj_filler(NSTRIP - 1, ytn_strips[NSTRIP - 1],
                                   evict_alt=True):
                fn()

    nc.compile()
    return nc


_NC_CACHE = None


def _get_nc():
    global _NC_CACHE
    if _NC_CACHE is None:
        _NC_CACHE = _build_kernel()
    return _NC_CACHE


def _prep_inputs(x, wq, wk, wv, wo):
    """Host-side shard + layout prep. Returns per-core input maps."""
    import ml_dtypes
    bf16 = ml_dtypes.bfloat16
    f8 = ml_dtypes.float8_e4m3

    x = np.asarray(x, dtype=np.float32).reshape(T, D)
    wq = np.asarray(wq, dtype=np.float32)
    wk = np.asarray(wk, dtype=np.float32)
    wv = np.asarray(wv, dtype=np.float32)
    wo = np.asarray(wo, dtype=np.float32)

    xT_f = np.ascontiguousarray(x.T)
    xT_b = xT_f[:, 0:S].astype(bf16)
    xT_8 = xT_f.astype(f8)

    # head-dim permutation for rope: [even pair comps | odd pair comps]
    perm = np.concatenate([np.arange(0, HD, 2), np.arange(1, HD, 2)])

    # rope tables in the [d, t] layout
    theta = 1.0 / ROPE_BASE ** (np.arange(0, HD, 2, dtype=np.float64) / HD)
    ang = np.arange(T, dtype=np.float64)[None, :] * theta[:, None]  # [32, T]
    cos_blk = np.cos(ang).astype(np.float32)
    sin_blk = np.sin(ang).astype(np.float32)
    cosT = np.tile(np.concatenate([cos_blk, cos_blk], 0), (2, 1))
    sinT = np.tile(np.concatenate([sin_blk, -sin_blk], 0), (2, 1))
    cosT = np.ascontiguousarray(cosT)
    sinT = np.ascontiguousarray(sinT)

    in_maps = []
    for c in range(NCORES):
        wq_c = wq[c * QC:(c + 1) * QC].reshape(HPC, HD, D)[:, perm, :]
        wq_c = wq_c.reshape(QC, D)
        wk_c = wk[c * HD:(c + 1) * HD][perm, :]
        wv_c = wv[c * HD:(c + 1) * HD]
        wkv_c = np.concatenate([wk_c, wv_c], axis=0)          # [128, D]
        wo_c = wo[:, c * QC:(c + 1) * QC]                      # [D, QC]
        wqT_c = np.ascontiguousarray(wq_c.T)
        wkvT_c = np.ascontiguousarray(wkv_c.T)
        woT_c = np.ascontiguousarray(wo_c.T)
        in_maps.append({
            "xT": xT_b,
            "xT8": xT_8,
            "wqT": wqT_c.astype(bf16),
            "wq8T": wqT_c.astype(f8),
            "wkvT": wkvT_c.astype(bf16),
            "wkv8T": wkvT_c.astype(f8),
            "woT": woT_c.astype(bf16),
            "wo8T": woT_c.astype(f8),
            "cosT": cosT.astype(bf16),
            "sinT": sinT.astype(bf16),
        })
    return in_maps


def kernel(x, wq, wk, wv, wo):
    from concourse.bass_utils import run_bass_kernel_spmd

    nc = _get_nc()
    in_maps = _prep_inputs(x, wq, wk, wv, wo)
    res = run_bass_kernel_spmd(nc, in_maps, core_ids=list(range(NCORES)))
    acc = np.zeros((T, D), dtype=np.float64)
    for c in range(NCORES):
        acc += res.results[c]["out"].astype(np.float64)
    return acc.astype(np.float32).reshape(1, T, D)
